# revision 20
# baseline (speedup 1.0000x reference)
"""Trainium2 Bass kernel for the PhaseODEFunc problem.

Math (reference):
    diff = phi[:,None,:] - xi[None,:,:]                 # (B, P, N)
    m = cos(diff).sum(-1)                               # (B, P)
    w = softmax(BETA * m / N, axis=-1)                  # (B, P)
    coupling = einsum('bp,bpn->bn', w, sin(diff))       # (B, N)
    dphi = K*coupling + A*sin(omega*t - phi)            # (B, N)

Implementation: angle-addition identities turn the O(B*P*N) trig work into
4 small matmuls plus O((B+P)*N) sin/cos evaluations:
    m  = cos(phi) @ cos(xi).T + sin(phi) @ sin(xi).T
    coupling = sin(phi) . (w @ cos(xi)) - cos(phi) . (w @ sin(xi))
    sin(wt - phi) = sin(wt)cos(phi) - cos(wt)sin(phi)
so  dphi = sin(phi).(wc - A cos(wt)) - cos(phi).(ws - A sin(wt))
with wc = w @ cos(xi), ws = w @ sin(xi).

ACT's Sin table is only valid on [-pi, pi]; angles are in [0, 2pi):
  - sin side: sin(x - pi) = -sin(x), pure in-range bias; the sign cancels
    in the sin.sin' matmul and is fixed up in the tail combine.
  - cos side: one ADD_RANGE_WRAP (shift pi/2, wrap to [-pi, pi]) per angle.

The logits matmul contracts over N, so it needs n-on-partitions layouts of
the trig values.  Those are produced in bf16 by the DMA xbar transpose
(SBUF->SBUF, off the compute engines); the matmuls then run in bf16
(single-pass, vs multi-pass fp32 on the PE).  The softmax stage is tiny
and replicated on all 8 cores; output columns are sharded 512/core.  The
coupling matmuls contract over P=256 with fp32r (fp32 rounded to 12
mantissa bits) inputs for accuracy.
"""

import math

import numpy as np

import concourse.bass as bass
import concourse.bacc as bacc
import concourse.tile as tile
import concourse.mybir as mybir
import concourse.bass_utils as bass_utils
from concourse.bass import ts, ds
from concourse.masks import make_identity

TWO_PI = 2.0 * math.pi
BETA = 1.0
K_COUP = 1.0
A_ANC = 0.08
OMEGA_ANC = TWO_PI * 200.0
B, P, N = 64, 256, 4096

N_CORES = 8
NLOC = N // N_CORES          # 512 output columns per core
NCH = 128                    # contraction chunk (partition dim)
NCHUNKS = N // NCH           # 32
NGR = 4                      # pipeline groups over the N dimension
GCOLS = N // NGR             # 1024 columns per group
GCH = GCOLS // NCH           # 8 chunks per group

F32 = mybir.dt.float32
F32R = mybir.dt.float32r
BF16 = mybir.dt.bfloat16
U16 = mybir.dt.uint16

_cached = {}


def _build(trace_sim=False):
    nc = bacc.Bacc("TRN2", target_bir_lowering=False, debug=False,
                   num_devices=N_CORES)
    phi_d = nc.dram_tensor("phi", [B, N], F32, kind="ExternalInput")
    xi_d = nc.dram_tensor("xi", [P, N], F32, kind="ExternalInput")
    phil_d = nc.dram_tensor("phi_l", [B, NLOC], F32, kind="ExternalInput")
    xil_d = nc.dram_tensor("xi_l", [P, NLOC], F32, kind="ExternalInput")
    anc_d = nc.dram_tensor("anc", [1, 2], F32, kind="ExternalInput")
    out_d = nc.dram_tensor("dphi", [B, NLOC], F32, kind="ExternalOutput")

    from contextlib import ExitStack
    with tile.TileContext(nc, trace_sim=trace_sim) as tc:
        with ExitStack() as ctx:
            _emit(tc, phi_d.ap(), xi_d.ap(), phil_d.ap(), xil_d.ap(),
                  anc_d.ap(), out_d.ap(), ctx)
    nc.compile()
    return nc


def _emit(tc, phi, xi, phi_l, xi_l, anc, out, ctx):
    nc = tc.nc
    SIN = mybir.ActivationFunctionType.Sin
    EXP = mybir.ActivationFunctionType.Exp
    HALF_PI = math.pi / 2.0

    consts = ctx.enter_context(tc.tile_pool(name="consts", bufs=1))
    negpi = consts.tile([128, 1], F32)
    nc.vector.memset(negpi, -math.pi)
    ident = consts.tile([B, B], F32)
    anc_sb = consts.tile([B, 2], F32)

    sb = ctx.enter_context(tc.tile_pool(name="sb", bufs=1))

    # ---- local (output-slice) inputs + natural-layout trig --------------
    xi_l_r = xi_l.rearrange("(t p) n -> p t n", p=128)       # [128, 2, 512]
    xil_sb = sb.tile([128, 2, NLOC], F32)
    nc.sync.dma_start(out=xil_sb, in_=xi_l_r)
    phil_sb = sb.tile([B, NLOC], F32)
    nc.sync.dma_start(out=phil_sb, in_=phi_l)

    # nsxi_l/nsphi_l hold sin(x - pi) = -sin(x); signs cancel downstream.
    cxi_l = sb.tile([128, 2, NLOC], F32R)
    nsxi_l = sb.tile([128, 2, NLOC], F32R)
    nc.scalar.activation(nsxi_l, xil_sb, SIN, bias=negpi)
    nc.vector.add_range_wrap(xil_sb, xil_sb, HALF_PI, math.pi, TWO_PI)
    nc.scalar.activation(cxi_l, xil_sb, SIN)
    cphi_l = sb.tile([B, NLOC], F32)
    nsphi_l = sb.tile([B, NLOC], F32)
    nc.scalar.activation(nsphi_l, phil_sb, SIN, bias=negpi[:B])
    nc.vector.add_range_wrap(phil_sb, phil_sb, HALF_PI, math.pi, TWO_PI)
    nc.scalar.activation(cphi_l, phil_sb, SIN)

    # ---- full inputs (replicated): load f32, downcast to bf16 on DVE,
    # DMA-xbar-transpose the bf16 ANGLES (half the transpose volume of
    # transposing trig values), then trig directly on the transposed
    # layout: sin via the -pi bias, cos via ADD_RANGE_WRAP.
    # Per-group pool tiles give Tile precise deps + double buffering. ----
    xi_r = xi.rearrange("(t p) n -> p t n", p=128)           # [128, 2, 4096]

    mm_ps = ctx.enter_context(tc.tile_pool(name="mm_ps", bufs=1, space="PSUM"))
    m_ps = mm_ps.tile([B, P], F32)

    ldp = ctx.enter_context(tc.tile_pool(name="ldp", bufs=3))
    angp = ctx.enter_context(tc.tile_pool(name="angp", bufs=3))
    trigp = ctx.enter_context(tc.tile_pool(name="trigp", bufs=2))

    for g in range(NGR):
        gsl = ts(g, GCOLS)
        xi_f = ldp.tile([128, 2, GCOLS], F32, name="xi_f")
        phi_f = ldp.tile([B, GCOLS], F32, name="phi_f")
        nc.gpsimd.dma_start(out=xi_f, in_=xi_r[:, :, gsl])
        nc.gpsimd.dma_start(out=phi_f, in_=phi[:, gsl])
        xi_bf = ldp.tile([128, 2, GCOLS], BF16, name="xi_bf")
        phi_bf = ldp.tile([B, GCOLS], BF16, name="phi_bf")
        nc.vector.tensor_copy(xi_bf, xi_f)
        nc.vector.tensor_copy(phi_bf, phi_f)

        # transposed angles: [n % 128, chunk, row]
        xiT = angp.tile([128, GCH, P], BF16, name="xiT")
        phiT = angp.tile([128, GCH, B], BF16, name="phiT")
        for t in range(2):
            nc.sync.dma_start_transpose(xiT[:, :, ts(t, 128)], xi_bf[:, t, :])
        nc.sync.dma_start_transpose(phiT, phi_bf)

        wxiT = angp.tile([128, GCH, P], BF16, name="wxiT")
        wphiT = angp.tile([128, GCH, B], BF16, name="wphiT")
        nc.vector.add_range_wrap(wxiT, xiT, HALF_PI, math.pi, TWO_PI)
        nc.vector.add_range_wrap(wphiT, phiT, HALF_PI, math.pi, TWO_PI)

        nsxiT = trigp.tile([128, GCH, P], BF16, name="nsxiT")
        nsphiT = trigp.tile([128, GCH, B], BF16, name="nsphiT")
        cxiT = trigp.tile([128, GCH, P], BF16, name="cxiT")
        cphiT = trigp.tile([128, GCH, B], BF16, name="cphiT")
        nc.scalar.activation(nsxiT, xiT, SIN, bias=negpi)
        nc.scalar.activation(nsphiT, phiT, SIN, bias=negpi)
        nc.scalar.activation(cxiT, wxiT, SIN)
        nc.scalar.activation(cphiT, wphiT, SIN)

        for j in range(GCH):
            k = g * GCH + j
            nc.tensor.matmul(
                m_ps, lhsT=cphiT[:, j, :], rhs=cxiT[:, j, :],
                start=(k == 0), stop=False, skip_group_check=True)
            nc.tensor.matmul(
                m_ps, lhsT=nsphiT[:, j, :], rhs=nsxiT[:, j, :],
                start=False, stop=(k == NCHUNKS - 1), skip_group_check=True)

    # ---- softmax over p (replicated; logits = m / N, no max needed:
    # |m|/N <= 1 so exp is safe and matches softmax exactly after norm) ---
    wexp = sb.tile([B, P], F32)
    ssum = sb.tile([B, 1], F32)
    nc.scalar.activation(wexp, m_ps, EXP, scale=BETA / N, accum_out=ssum)
    rinv = sb.tile([B, 1], F32)
    nc.vector.reciprocal(rinv, ssum)
    w_sb = sb.tile([B, P], F32)
    nc.vector.tensor_scalar_mul(w_sb, in0=wexp, scalar1=rinv)

    make_identity(nc, ident)
    nc.sync.dma_start(out=anc_sb, in_=anc.to_broadcast((B, 2)))

    with tc.tile_pool(name="tail_ps", bufs=1, space="PSUM") as tail_ps:
        wt_ps = tail_ps.tile([128, 2, B], F32)
        for h in range(2):
            nc.tensor.transpose(wt_ps[:, h, :], w_sb[:, ts(h, 128)], ident)
        wT = sb.tile([128, 2, B], F32R)
        nc.vector.tensor_copy(wT, wt_ps)

        # ---- coupling on the local slice: wc = w @ cxi, -ws = w @ -sxi --
        wc_ps = tail_ps.tile([B, NLOC], F32)
        ws_ps = tail_ps.tile([B, NLOC], F32)
        for h in range(2):
            nc.tensor.matmul(wc_ps, lhsT=wT[:, h, :], rhs=cxi_l[:, h, :],
                             start=(h == 0), stop=(h == 1),
                             skip_group_check=True)
            nc.tensor.matmul(ws_ps, lhsT=wT[:, h, :], rhs=nsxi_l[:, h, :],
                             start=(h == 0), stop=(h == 1),
                             skip_group_check=True)

        # q1 = (wc - A cos u) * (-sin phi);  ws_ps holds -ws, so
        # q2 = (-ws + A sin u) * cos phi;  dphi = q2 - q1
        q1 = sb.tile([B, NLOC], F32)
        q2 = sb.tile([B, NLOC], F32)
        nc.vector.scalar_tensor_tensor(
            q1, in0=wc_ps, scalar=anc_sb[:, 1:2], in1=nsphi_l,
            op0=mybir.AluOpType.subtract, op1=mybir.AluOpType.mult)
        nc.vector.scalar_tensor_tensor(
            q2, in0=ws_ps, scalar=anc_sb[:, 0:1], in1=cphi_l,
            op0=mybir.AluOpType.add, op1=mybir.AluOpType.mult)
        dphi_sb = sb.tile([B, NLOC], F32)
        nc.vector.tensor_sub(dphi_sb, q2, q1)
        nc.sync.dma_start(out=out, in_=dphi_sb)


def kernel(t, phi, xi):
    t = np.asarray(t, dtype=np.float32)
    phi = np.ascontiguousarray(np.asarray(phi, dtype=np.float32))
    xi = np.ascontiguousarray(np.asarray(xi, dtype=np.float32))

    if "nc" not in _cached:
        _cached["nc"] = _build()
    nc = _cached["nc"]

    # anchor phase: match reference f32 rounding of omega*t, then take
    # sin/cos at f64 accuracy
    u = np.float32(OMEGA_ANC) * np.float32(t.reshape(-1)[0])
    anc = np.array([[A_ANC * math.sin(float(u)),
                     A_ANC * math.cos(float(u))]], dtype=np.float32)

    in_maps = []
    for c in range(N_CORES):
        sl = slice(c * NLOC, (c + 1) * NLOC)
        in_maps.append({
            "phi": phi,
            "xi": xi,
            "phi_l": np.ascontiguousarray(phi[:, sl]),
            "xi_l": np.ascontiguousarray(xi[:, sl]),
            "anc": anc,
        })

    res = bass_utils.run_bass_kernel_spmd(
        nc, in_maps, core_ids=list(range(N_CORES)))
    return np.concatenate([res.results[c]["dphi"] for c in range(N_CORES)],
                          axis=1)


# revision 21
# speedup vs baseline: 1.6223x; 1.6223x over previous
"""Trainium2 Bass kernel for the PhaseODEFunc problem.

Math (reference):
    diff = phi[:,None,:] - xi[None,:,:]                 # (B, P, N)
    m = cos(diff).sum(-1)                               # (B, P)
    w = softmax(BETA * m / N, axis=-1)                  # (B, P)
    coupling = einsum('bp,bpn->bn', w, sin(diff))       # (B, N)
    dphi = K*coupling + A*sin(omega*t - phi)            # (B, N)

Implementation: angle-addition identities turn the O(B*P*N) trig work into
4 small matmuls plus O((B+P)*N) sin/cos evaluations:
    m  = cos(phi) @ cos(xi).T + sin(phi) @ sin(xi).T
    coupling = sin(phi) . (w @ cos(xi)) - cos(phi) . (w @ sin(xi))
    sin(wt - phi) = sin(wt)cos(phi) - cos(wt)sin(phi)
so  dphi = sin(phi).(wc - A cos(wt)) - cos(phi).(ws - A sin(wt))
with wc = w @ cos(xi), ws = w @ sin(xi).

ACT's Sin table is only valid on [-pi, pi]; angles are in [0, 2pi):
  - sin side: sin(x - pi) = -sin(x), pure in-range bias; the sign cancels
    in the sin.sin' matmul and is fixed up in the tail combine.
  - cos side: one ADD_RANGE_WRAP (shift pi/2, wrap to [-pi, pi]) per angle.

The logits matmul contracts over N, so it needs n-on-partitions layouts of
the trig values.  Those are produced in bf16 by the DMA xbar transpose
(SBUF->SBUF, off the compute engines); the matmuls then run in bf16
(single-pass, vs multi-pass fp32 on the PE).  The softmax stage is tiny
and replicated on all 8 cores; output columns are sharded 512/core.  The
coupling matmuls contract over P=256 with fp32r (fp32 rounded to 12
mantissa bits) inputs for accuracy.
"""

import math

import numpy as np

import concourse.bass as bass
import concourse.bacc as bacc
import concourse.tile as tile
import concourse.mybir as mybir
import concourse.bass_utils as bass_utils
from concourse.bass import ts, ds
from concourse.masks import make_identity

TWO_PI = 2.0 * math.pi
BETA = 1.0
K_COUP = 1.0
A_ANC = 0.08
OMEGA_ANC = TWO_PI * 200.0
B, P, N = 64, 256, 4096

N_CORES = 8
NLOC = N // N_CORES          # 512 output columns per core
NCH = 128                    # contraction chunk (partition dim)
NCHUNKS = N // NCH           # 32
NGR = 8                      # pipeline groups over the N dimension
GCOLS = N // NGR             # 512 columns per group
GCH = GCOLS // NCH           # 4 chunks per group

F32 = mybir.dt.float32
F32R = mybir.dt.float32r
BF16 = mybir.dt.bfloat16
U16 = mybir.dt.uint16

_cached = {}


def _build(trace_sim=False):
    nc = bacc.Bacc("TRN2", target_bir_lowering=False, debug=False,
                   num_devices=N_CORES)
    phi_d = nc.dram_tensor("phi", [B, N], F32, kind="ExternalInput")
    xi_d = nc.dram_tensor("xi", [P, N], F32, kind="ExternalInput")
    phil_d = nc.dram_tensor("phi_l", [B, NLOC], F32, kind="ExternalInput")
    xil_d = nc.dram_tensor("xi_l", [P, NLOC], F32, kind="ExternalInput")
    anc_d = nc.dram_tensor("anc", [1, 2], F32, kind="ExternalInput")
    out_d = nc.dram_tensor("dphi", [B, NLOC], F32, kind="ExternalOutput")

    from contextlib import ExitStack
    with tile.TileContext(nc, trace_sim=trace_sim) as tc:
        with ExitStack() as ctx:
            _emit(tc, phi_d.ap(), xi_d.ap(), phil_d.ap(), xil_d.ap(),
                  anc_d.ap(), out_d.ap(), ctx)
    nc.compile()
    return nc


def _emit(tc, phi, xi, phi_l, xi_l, anc, out, ctx):
    nc = tc.nc
    SIN = mybir.ActivationFunctionType.Sin
    EXP = mybir.ActivationFunctionType.Exp
    HALF_PI = math.pi / 2.0

    consts = ctx.enter_context(tc.tile_pool(name="consts", bufs=1))
    negpi = consts.tile([128, 1], F32)
    nc.vector.memset(negpi, -math.pi)
    identbf = consts.tile([128, 128], BF16)
    make_identity(nc, identbf)
    ident = consts.tile([B, B], F32)
    anc_sb = consts.tile([B, 2], F32)

    sb = ctx.enter_context(tc.tile_pool(name="sb", bufs=1))

    # ---- local (output-slice) inputs + natural-layout trig --------------
    xi_l_r = xi_l.rearrange("(t p) n -> p t n", p=128)       # [128, 2, 512]
    xil_sb = sb.tile([128, 2, NLOC], F32)
    nc.sync.dma_start(out=xil_sb, in_=xi_l_r)
    phil_sb = sb.tile([B, NLOC], F32)
    nc.sync.dma_start(out=phil_sb, in_=phi_l)

    # nsxi_l/nsphi_l hold sin(x - pi) = -sin(x); signs cancel downstream.
    cxi_l = sb.tile([128, 2, NLOC], F32R)
    nsxi_l = sb.tile([128, 2, NLOC], F32R)
    nc.scalar.activation(nsxi_l, xil_sb, SIN, bias=negpi)
    nc.vector.add_range_wrap(xil_sb, xil_sb, HALF_PI, math.pi, TWO_PI)
    nc.scalar.activation(cxi_l, xil_sb, SIN)
    cphi_l = sb.tile([B, NLOC], F32)
    nsphi_l = sb.tile([B, NLOC], F32)
    nc.scalar.activation(nsphi_l, phil_sb, SIN, bias=negpi[:B])
    nc.vector.add_range_wrap(phil_sb, phil_sb, HALF_PI, math.pi, TWO_PI)
    nc.scalar.activation(cphi_l, phil_sb, SIN)

    # ---- full inputs (replicated): load f32, downcast to bf16 on DVE,
    # DMA-xbar-transpose the bf16 ANGLES (half the transpose volume of
    # transposing trig values), then trig directly on the transposed
    # layout: sin via the -pi bias, cos via ADD_RANGE_WRAP.
    # Per-group pool tiles give Tile precise deps + double buffering. ----
    xi_r = xi.rearrange("(t p) n -> p t n", p=128)           # [128, 2, 4096]

    mm_ps = ctx.enter_context(tc.tile_pool(name="mm_ps", bufs=1, space="PSUM"))
    m_ps = mm_ps.tile([B, P], F32)

    ldp = ctx.enter_context(tc.tile_pool(name="ldp", bufs=3))
    stgp = ctx.enter_context(tc.tile_pool(name="stgp", bufs=2, space="PSUM"))
    angp = ctx.enter_context(tc.tile_pool(name="angp", bufs=2))
    trigp = ctx.enter_context(tc.tile_pool(name="trigp", bufs=2))

    for g in range(NGR):
        gsl = ts(g, GCOLS)
        xi_f = ldp.tile([128, 2, GCOLS], F32, name="xi_f")
        phi_f = ldp.tile([B, GCOLS], F32, name="phi_f")
        nc.gpsimd.dma_start(out=xi_f, in_=xi_r[:, :, gsl])
        nc.gpsimd.dma_start(out=phi_f, in_=phi[:, gsl])
        xi_bf = ldp.tile([128, 2, GCOLS], BF16, name="xi_bf")
        phi_bf = ldp.tile([B, GCOLS], BF16, name="phi_bf")
        nc.vector.tensor_copy(xi_bf, xi_f)
        nc.gpsimd.tensor_copy(phi_bf, phi_f)

        # PE-transpose bf16 angle tiles into PSUM staging.  Each chunk block
        # is padded to 512 bf16 columns (one half-bank alignment) so every
        # transpose write stays inside a single 2KB PSUM bank:
        # [0:128]=xiT(p<128), [128:256]=xiT(p>=128), [256:320]=phiT.
        stg = stgp.tile([128, GCH, 512], BF16, name="stg")
        for j in range(GCH):
            ksl = ts(g * GCH + j, NCH)
            lsl = ts(j, NCH)
            nc.tensor.transpose(stg[:, j, 0:128], xi_bf[:, 0, lsl], identbf)
            nc.tensor.transpose(stg[:, j, 128:256], xi_bf[:, 1, lsl], identbf)
            nc.tensor.transpose(stg[:, j, 256:320], phi_bf[:, lsl],
                                identbf[:B, :B])
        stg_v = stg[:, :, 0:320]

        # cos-side angles: range-wrap from PSUM staging into SBUF
        wT = angp.tile([128, GCH, 320], BF16, name="wT")
        nc.vector.add_range_wrap(wT, stg_v, HALF_PI, math.pi, TWO_PI)

        trig_ns = trigp.tile([128, GCH, 320], BF16, name="trig_ns")
        trig_c = trigp.tile([128, GCH, 320], BF16, name="trig_c")
        nc.scalar.activation(trig_ns, stg_v, SIN, bias=negpi)
        nc.scalar.activation(trig_c, wT, SIN)

        for j in range(GCH):
            k = g * GCH + j
            nc.tensor.matmul(
                m_ps, lhsT=trig_c[:, j, 256:320], rhs=trig_c[:, j, 0:256],
                start=(k == 0), stop=False, skip_group_check=True)
            nc.tensor.matmul(
                m_ps, lhsT=trig_ns[:, j, 256:320], rhs=trig_ns[:, j, 0:256],
                start=False, stop=(k == NCHUNKS - 1), skip_group_check=True)

    # ---- softmax over p (replicated; logits = m / N, no max needed:
    # |m|/N <= 1 so exp is safe and matches softmax exactly after norm) ---
    wexp = sb.tile([B, P], F32)
    ssum = sb.tile([B, 1], F32)
    nc.scalar.activation(wexp, m_ps, EXP, scale=BETA / N, accum_out=ssum)
    rinv = sb.tile([B, 1], F32)
    nc.vector.reciprocal(rinv, ssum)
    w_sb = sb.tile([B, P], F32)
    nc.vector.tensor_scalar_mul(w_sb, in0=wexp, scalar1=rinv)

    make_identity(nc, ident)
    nc.sync.dma_start(out=anc_sb, in_=anc.to_broadcast((B, 2)))

    with tc.tile_pool(name="tail_ps", bufs=1, space="PSUM") as tail_ps:
        wt_ps = tail_ps.tile([128, 2, B], F32)
        for h in range(2):
            nc.tensor.transpose(wt_ps[:, h, :], w_sb[:, ts(h, 128)], ident)
        wT = sb.tile([128, 2, B], F32R)
        nc.vector.tensor_copy(wT, wt_ps)

        # ---- coupling on the local slice: wc = w @ cxi, -ws = w @ -sxi --
        wc_ps = tail_ps.tile([B, NLOC], F32)
        ws_ps = tail_ps.tile([B, NLOC], F32)
        for h in range(2):
            nc.tensor.matmul(wc_ps, lhsT=wT[:, h, :], rhs=cxi_l[:, h, :],
                             start=(h == 0), stop=(h == 1),
                             skip_group_check=True)
            nc.tensor.matmul(ws_ps, lhsT=wT[:, h, :], rhs=nsxi_l[:, h, :],
                             start=(h == 0), stop=(h == 1),
                             skip_group_check=True)

        # q1 = (wc - A cos u) * (-sin phi);  ws_ps holds -ws, so
        # q2 = (-ws + A sin u) * cos phi;  dphi = q2 - q1
        q1 = sb.tile([B, NLOC], F32)
        q2 = sb.tile([B, NLOC], F32)
        nc.vector.scalar_tensor_tensor(
            q1, in0=wc_ps, scalar=anc_sb[:, 1:2], in1=nsphi_l,
            op0=mybir.AluOpType.subtract, op1=mybir.AluOpType.mult)
        nc.vector.scalar_tensor_tensor(
            q2, in0=ws_ps, scalar=anc_sb[:, 0:1], in1=cphi_l,
            op0=mybir.AluOpType.add, op1=mybir.AluOpType.mult)
        dphi_sb = sb.tile([B, NLOC], F32)
        nc.vector.tensor_sub(dphi_sb, q2, q1)
        nc.sync.dma_start(out=out, in_=dphi_sb)


def kernel(t, phi, xi):
    t = np.asarray(t, dtype=np.float32)
    phi = np.ascontiguousarray(np.asarray(phi, dtype=np.float32))
    xi = np.ascontiguousarray(np.asarray(xi, dtype=np.float32))

    if "nc" not in _cached:
        _cached["nc"] = _build()
    nc = _cached["nc"]

    # anchor phase: match reference f32 rounding of omega*t, then take
    # sin/cos at f64 accuracy
    u = np.float32(OMEGA_ANC) * np.float32(t.reshape(-1)[0])
    anc = np.array([[A_ANC * math.sin(float(u)),
                     A_ANC * math.cos(float(u))]], dtype=np.float32)

    in_maps = []
    for c in range(N_CORES):
        sl = slice(c * NLOC, (c + 1) * NLOC)
        in_maps.append({
            "phi": phi,
            "xi": xi,
            "phi_l": np.ascontiguousarray(phi[:, sl]),
            "xi_l": np.ascontiguousarray(xi[:, sl]),
            "anc": anc,
        })

    res = bass_utils.run_bass_kernel_spmd(
        nc, in_maps, core_ids=list(range(N_CORES)))
    return np.concatenate([res.results[c]["dphi"] for c in range(N_CORES)],
                          axis=1)


# revision 23
# speedup vs baseline: 1.6487x; 1.0163x over previous
"""Trainium2 Bass kernel for the PhaseODEFunc problem.

Math (reference):
    diff = phi[:,None,:] - xi[None,:,:]                 # (B, P, N)
    m = cos(diff).sum(-1)                               # (B, P)
    w = softmax(BETA * m / N, axis=-1)                  # (B, P)
    coupling = einsum('bp,bpn->bn', w, sin(diff))       # (B, N)
    dphi = K*coupling + A*sin(omega*t - phi)            # (B, N)

Implementation: angle-addition identities turn the O(B*P*N) trig work into
4 small matmuls plus O((B+P)*N) sin/cos evaluations:
    m  = cos(phi) @ cos(xi).T + sin(phi) @ sin(xi).T
    coupling = sin(phi) . (w @ cos(xi)) - cos(phi) . (w @ sin(xi))
    sin(wt - phi) = sin(wt)cos(phi) - cos(wt)sin(phi)
so  dphi = sin(phi).(wc - A cos(wt)) - cos(phi).(ws - A sin(wt))
with wc = w @ cos(xi), ws = w @ sin(xi).

ACT's Sin table is only valid on [-pi, pi]; angles are in [0, 2pi):
  - sin side: sin(x - pi) = -sin(x), pure in-range bias; the sign cancels
    in the sin.sin' matmul and is fixed up in the tail combine.
  - cos side: one ADD_RANGE_WRAP (shift pi/2, wrap to [-pi, pi]) per angle.

The logits matmul contracts over N, so it needs n-on-partitions layouts of
the trig values.  Those are produced in bf16 by the DMA xbar transpose
(SBUF->SBUF, off the compute engines); the matmuls then run in bf16
(single-pass, vs multi-pass fp32 on the PE).  The softmax stage is tiny
and replicated on all 8 cores; output columns are sharded 512/core.  The
coupling matmuls contract over P=256 with fp32r (fp32 rounded to 12
mantissa bits) inputs for accuracy.
"""

import math

import numpy as np

import concourse.bass as bass
import concourse.bacc as bacc
import concourse.tile as tile
import concourse.mybir as mybir
import concourse.bass_utils as bass_utils
from concourse.bass import ts, ds
from concourse.masks import make_identity

TWO_PI = 2.0 * math.pi
BETA = 1.0
K_COUP = 1.0
A_ANC = 0.08
OMEGA_ANC = TWO_PI * 200.0
B, P, N = 64, 256, 4096

N_CORES = 8
NLOC = N // N_CORES          # 512 output columns per core
NCH = 128                    # contraction chunk (partition dim)
NCHUNKS = N // NCH           # 32
NGR = 8                      # pipeline groups over the N dimension
GCOLS = N // NGR             # 512 columns per group
GCH = GCOLS // NCH           # 4 chunks per group

F32 = mybir.dt.float32
F32R = mybir.dt.float32r
BF16 = mybir.dt.bfloat16
U16 = mybir.dt.uint16

_cached = {}


def _build(trace_sim=False):
    nc = bacc.Bacc("TRN2", target_bir_lowering=False, debug=False,
                   num_devices=N_CORES)
    phi_d = nc.dram_tensor("phi", [B, N], F32, kind="ExternalInput")
    xi_d = nc.dram_tensor("xi", [P, N], F32, kind="ExternalInput")
    phil_d = nc.dram_tensor("phi_l", [B, NLOC], F32, kind="ExternalInput")
    xil_d = nc.dram_tensor("xi_l", [P, NLOC], F32, kind="ExternalInput")
    anc_d = nc.dram_tensor("anc", [1, 2], F32, kind="ExternalInput")
    out_d = nc.dram_tensor("dphi", [B, NLOC], F32, kind="ExternalOutput")

    from contextlib import ExitStack
    with tile.TileContext(nc, trace_sim=trace_sim) as tc:
        with ExitStack() as ctx:
            _emit(tc, phi_d.ap(), xi_d.ap(), phil_d.ap(), xil_d.ap(),
                  anc_d.ap(), out_d.ap(), ctx)
    nc.compile()
    return nc


def _emit(tc, phi, xi, phi_l, xi_l, anc, out, ctx):
    nc = tc.nc
    SIN = mybir.ActivationFunctionType.Sin
    EXP = mybir.ActivationFunctionType.Exp
    HALF_PI = math.pi / 2.0

    consts = ctx.enter_context(tc.tile_pool(name="consts", bufs=1))
    negpi = consts.tile([128, 1], F32)
    nc.vector.memset(negpi, -math.pi)
    identbf = consts.tile([128, 128], BF16)
    make_identity(nc, identbf)
    ident = consts.tile([B, B], F32)
    anc_sb = consts.tile([B, 2], F32)

    sb = ctx.enter_context(tc.tile_pool(name="sb", bufs=1))

    # ---- local (output-slice) inputs + natural-layout trig --------------
    xi_l_r = xi_l.rearrange("(t p) n -> p t n", p=128)       # [128, 2, 512]
    xil_sb = sb.tile([128, 2, NLOC], F32)
    nc.sync.dma_start(out=xil_sb, in_=xi_l_r)
    phil_sb = sb.tile([B, NLOC], F32)
    nc.sync.dma_start(out=phil_sb, in_=phi_l)

    # nsxi_l/nsphi_l hold sin(x - pi) = -sin(x); signs cancel downstream.
    cxi_l = sb.tile([128, 2, NLOC], F32R)
    nsxi_l = sb.tile([128, 2, NLOC], F32R)
    nc.scalar.activation(nsxi_l, xil_sb, SIN, bias=negpi)
    nc.vector.add_range_wrap(xil_sb, xil_sb, HALF_PI, math.pi, TWO_PI)
    nc.scalar.activation(cxi_l, xil_sb, SIN)
    cphi_l = sb.tile([B, NLOC], F32)
    nsphi_l = sb.tile([B, NLOC], F32)
    nc.scalar.activation(nsphi_l, phil_sb, SIN, bias=negpi[:B])
    nc.vector.add_range_wrap(phil_sb, phil_sb, HALF_PI, math.pi, TWO_PI)
    nc.scalar.activation(cphi_l, phil_sb, SIN)

    # ---- full inputs (replicated): load f32, downcast to bf16 on DVE,
    # DMA-xbar-transpose the bf16 ANGLES (half the transpose volume of
    # transposing trig values), then trig directly on the transposed
    # layout: sin via the -pi bias, cos via ADD_RANGE_WRAP.
    # Per-group pool tiles give Tile precise deps + double buffering. ----
    xi_r = xi.rearrange("(t p) n -> p t n", p=128)           # [128, 2, 4096]

    mm_ps = ctx.enter_context(tc.tile_pool(name="mm_ps", bufs=1, space="PSUM"))
    m_ps = mm_ps.tile([B, P], F32)

    from contextlib import ExitStack
    loop_ctx = ExitStack()
    ldp = loop_ctx.enter_context(tc.tile_pool(name="ldp", bufs=3))
    stgp = loop_ctx.enter_context(tc.tile_pool(name="stgp", bufs=3, space="PSUM"))
    angp = loop_ctx.enter_context(tc.tile_pool(name="angp", bufs=3))
    trigp = loop_ctx.enter_context(tc.tile_pool(name="trigp", bufs=3))

    for g in range(NGR):
        gsl = ts(g, GCOLS)
        xi_f = ldp.tile([128, 2, GCOLS], F32, name="xi_f")
        phi_f = ldp.tile([B, GCOLS], F32, name="phi_f")
        nc.sync.dma_start(out=xi_f, in_=xi_r[:, :, gsl])
        nc.gpsimd.dma_start(out=phi_f, in_=phi[:, gsl])
        xi_bf = ldp.tile([128, 2, GCOLS], BF16, name="xi_bf")
        phi_bf = ldp.tile([B, GCOLS], BF16, name="phi_bf")
        nc.vector.tensor_copy(xi_bf, xi_f)
        nc.gpsimd.tensor_copy(phi_bf, phi_f)

        # PE-transpose bf16 angle tiles into PSUM staging.  Each chunk block
        # is padded to 512 bf16 columns (one half-bank alignment) so every
        # transpose write stays inside a single 2KB PSUM bank:
        # [0:128]=xiT(p<128), [128:256]=xiT(p>=128), [256:320]=phiT.
        stg = stgp.tile([128, GCH, 512], BF16, name="stg")
        for j in range(GCH):
            ksl = ts(g * GCH + j, NCH)
            lsl = ts(j, NCH)
            nc.tensor.transpose(stg[:, j, 0:128], xi_bf[:, 0, lsl], identbf)
            nc.tensor.transpose(stg[:, j, 128:256], xi_bf[:, 1, lsl], identbf)
            nc.tensor.transpose(stg[:, j, 256:320], phi_bf[:, lsl],
                                identbf[:B, :B])
        stg_v = stg[:, :, 0:320]

        # cos-side angles: range-wrap from PSUM staging into SBUF
        wT = angp.tile([128, GCH, 320], BF16, name="wT")
        nc.vector.add_range_wrap(wT, stg_v, HALF_PI, math.pi, TWO_PI)

        trig_ns = trigp.tile([128, GCH, 320], BF16, name="trig_ns")
        trig_c = trigp.tile([128, GCH, 320], BF16, name="trig_c")
        nc.scalar.activation(trig_ns, stg_v, SIN, bias=negpi)
        nc.scalar.activation(trig_c, wT, SIN)

        for j in range(GCH):
            k = g * GCH + j
            nc.tensor.matmul(
                m_ps, lhsT=trig_c[:, j, 256:320], rhs=trig_c[:, j, 0:256],
                start=(k == 0), stop=False, skip_group_check=True)
            nc.tensor.matmul(
                m_ps, lhsT=trig_ns[:, j, 256:320], rhs=trig_ns[:, j, 0:256],
                start=False, stop=(k == NCHUNKS - 1), skip_group_check=True)

    # ---- softmax over p (replicated; logits = m / N, no max needed:
    # |m|/N <= 1 so exp is safe and matches softmax exactly after norm) ---
    wexp = sb.tile([B, P], F32)
    ssum = sb.tile([B, 1], F32)
    nc.scalar.activation(wexp, m_ps, EXP, scale=BETA / N, accum_out=ssum)
    rinv = sb.tile([B, 1], F32)
    nc.vector.reciprocal(rinv, ssum)
    w_sb = sb.tile([B, P], F32)
    nc.vector.tensor_scalar_mul(w_sb, in0=wexp, scalar1=rinv)

    loop_ctx.close()
    make_identity(nc, ident)
    nc.sync.dma_start(out=anc_sb, in_=anc.to_broadcast((B, 2)))

    with tc.tile_pool(name="tail_ps", bufs=1, space="PSUM") as tail_ps:
        wt_ps = tail_ps.tile([128, 2, B], F32)
        for h in range(2):
            nc.tensor.transpose(wt_ps[:, h, :], w_sb[:, ts(h, 128)], ident)
        wT = sb.tile([128, 2, B], F32R)
        nc.vector.tensor_copy(wT, wt_ps)

        # ---- coupling on the local slice: wc = w @ cxi, -ws = w @ -sxi --
        wc_ps = tail_ps.tile([B, NLOC], F32)
        ws_ps = tail_ps.tile([B, NLOC], F32)
        for h in range(2):
            nc.tensor.matmul(wc_ps, lhsT=wT[:, h, :], rhs=cxi_l[:, h, :],
                             start=(h == 0), stop=(h == 1),
                             skip_group_check=True)
            nc.tensor.matmul(ws_ps, lhsT=wT[:, h, :], rhs=nsxi_l[:, h, :],
                             start=(h == 0), stop=(h == 1),
                             skip_group_check=True)

        # q1 = (wc - A cos u) * (-sin phi);  ws_ps holds -ws, so
        # q2 = (-ws + A sin u) * cos phi;  dphi = q2 - q1
        q1 = sb.tile([B, NLOC], F32)
        q2 = sb.tile([B, NLOC], F32)
        nc.vector.scalar_tensor_tensor(
            q1, in0=wc_ps, scalar=anc_sb[:, 1:2], in1=nsphi_l,
            op0=mybir.AluOpType.subtract, op1=mybir.AluOpType.mult)
        nc.vector.scalar_tensor_tensor(
            q2, in0=ws_ps, scalar=anc_sb[:, 0:1], in1=cphi_l,
            op0=mybir.AluOpType.add, op1=mybir.AluOpType.mult)
        dphi_sb = sb.tile([B, NLOC], F32)
        nc.vector.tensor_sub(dphi_sb, q2, q1)
        nc.sync.dma_start(out=out, in_=dphi_sb)


def kernel(t, phi, xi):
    t = np.asarray(t, dtype=np.float32)
    phi = np.ascontiguousarray(np.asarray(phi, dtype=np.float32))
    xi = np.ascontiguousarray(np.asarray(xi, dtype=np.float32))

    if "nc" not in _cached:
        _cached["nc"] = _build()
    nc = _cached["nc"]

    # anchor phase: match reference f32 rounding of omega*t, then take
    # sin/cos at f64 accuracy
    u = np.float32(OMEGA_ANC) * np.float32(t.reshape(-1)[0])
    anc = np.array([[A_ANC * math.sin(float(u)),
                     A_ANC * math.cos(float(u))]], dtype=np.float32)

    in_maps = []
    for c in range(N_CORES):
        sl = slice(c * NLOC, (c + 1) * NLOC)
        in_maps.append({
            "phi": phi,
            "xi": xi,
            "phi_l": np.ascontiguousarray(phi[:, sl]),
            "xi_l": np.ascontiguousarray(xi[:, sl]),
            "anc": anc,
        })

    res = bass_utils.run_bass_kernel_spmd(
        nc, in_maps, core_ids=list(range(N_CORES)))
    return np.concatenate([res.results[c]["dphi"] for c in range(N_CORES)],
                          axis=1)
